# revision 1
# baseline (speedup 1.0000x reference)
"""DMPNNConv kernel for 8 Trainium2 NeuronCores.

  h_n = relu([x ; h_e] @ W_i_w.T + W_i_b)          [N, D]
  m   = einsum('kn,nd->d', bond_n, h_n)            [D]
  h   = relu(h_n + m @ W_m_w.T + W_m_b)            [N, D]

Sharding: N (edge dim) split 8 ways; weights replicated; single [D]
all-reduce of the message m between the two passes.

Device dataflow (per core, N_sh = 62976 padded rows = 123 tiles x 512):
  pass 1 (layout [d, t]):  load x/h_e tiles cast to bf16 (SWDGE cast-DMA),
    PE-transpose 128x128 blocks to put the feature axis on partitions,
    z.T = W1aT.T @ xT + W1bT.T @ heT  (PSUM f32), relu+bias on ACT -> bf16
    h_n tile, stored to DRAM scratch.  w broadcast = ones32.T @ bond (PE),
    m partial = sum_t h_n[d,t]*w[t] via DVE tensor_tensor_reduce.
  all-reduce m (512 B), c = W_m @ m + b2 on PE.
  pass 2: DMA-transpose scratch back to [t, d] (bf16, xbar), add broadcast
    c (DVE) + relu (ACT), store h.
"""

import os
import sys

sys.path.insert(0, "/opt/trn_rl_repo")

import numpy as np

N, D, K = 500000, 128, 32
CORES = 8
T = 512                      # tokens per tile
NB = T // 128                # 128-token blocks per tile
NT = 123                     # tiles per core
N_SH = NT * T                # 62976 padded rows per core
N_PAD = CORES * N_SH         # 503808

_cache = {}
last_results = None


def _build():
    import concourse.bass as bass
    import concourse.bacc as bacc
    import concourse.tile as tile
    import concourse.mybir as mybir
    from concourse import masks

    f32 = mybir.dt.float32
    bf16 = mybir.dt.bfloat16
    AF = mybir.ActivationFunctionType
    ALU = mybir.AluOpType

    nc = bacc.Bacc("TRN2", target_bir_lowering=False, debug=False,
                   num_devices=CORES)

    x_d = nc.dram_tensor("x", [N_SH, D], f32, kind="ExternalInput").ap()
    he_d = nc.dram_tensor("h_e", [N_SH, D], f32, kind="ExternalInput").ap()
    bond_d = nc.dram_tensor("bond_n", [K, N_SH], f32, kind="ExternalInput").ap()
    wi_d = nc.dram_tensor("W_i_w", [D, 2 * D], f32, kind="ExternalInput").ap()
    bi_d = nc.dram_tensor("W_i_b", [D], f32, kind="ExternalInput").ap()
    wm_d = nc.dram_tensor("W_m_w", [D, D], f32, kind="ExternalInput").ap()
    bm_d = nc.dram_tensor("W_m_b", [D], f32, kind="ExternalInput").ap()
    h_d = nc.dram_tensor("h", [N_SH, D], f32, kind="ExternalOutput").ap()

    x_re = x_d.rearrange("(i b p) d -> i p b d", b=NB, p=128)
    he_re = he_d.rearrange("(i b p) d -> i p b d", b=NB, p=128)
    h_re = h_d.rearrange("(i b p) d -> i p b d", b=NB, p=128)

    with tile.TileContext(nc) as tc:
        import contextlib
        ctx = contextlib.ExitStack()
        with ctx:
            pers = ctx.enter_context(tc.tile_pool(name="pers", bufs=1))
            io = ctx.enter_context(tc.tile_pool(name="io", bufs=6))
            ps_z = ctx.enter_context(tc.tile_pool(name="ps_z", bufs=2, space="PSUM"))
            ps_w = ctx.enter_context(tc.tile_pool(name="ps_w", bufs=2, space="PSUM"))
            ps_t = ctx.enter_context(tc.tile_pool(name="ps_t", bufs=4, space="PSUM"))
            dram = ctx.enter_context(tc.tile_pool(name="dram", bufs=1, space="DRAM"))

            # ---- one-time setup -------------------------------------------
            ident_bf = pers.tile([128, 128], bf16)
            masks.make_identity(nc, ident_bf[:])
            ident_f = pers.tile([128, 128], f32)
            masks.make_identity(nc, ident_f[:])

            ones32 = pers.tile([K, 128], bf16)
            nc.gpsimd.memset(ones32[:], 1.0)
            ones1 = pers.tile([1, 128], f32)
            nc.gpsimd.memset(ones1[:], 1.0)

            b1_col = pers.tile([128, 1], f32)
            nc.sync.dma_start(b1_col[:, 0], bi_d[:])
            b2_col = pers.tile([128, 1], f32)
            nc.sync.dma_start(b2_col[:, 0], bm_d[:])

            # W_i_w [D, 2D] -> bf16 -> transpose halves -> W1aT/W1bT [j, d]
            wi_sb = pers.tile([128, 2 * D], f32)
            nc.sync.dma_start(wi_sb[:], wi_d[:])
            wi_bf = pers.tile([128, 2 * D], bf16)
            nc.vector.tensor_copy(wi_bf[:], wi_sb[:])
            w1t = pers.tile([128, 2 * D], bf16)  # [j, (half d)]
            for half in range(2):
                tp = ps_t.tile([128, 128], bf16, tag="tr")
                nc.tensor.transpose(tp[:], wi_bf[:, 128 * half:128 * (half + 1)],
                                    ident_bf[:])
                nc.vector.tensor_copy(w1t[:, 128 * half:128 * (half + 1)], tp[:])

            # W_m_w [D, D] -> WmT [d, d'] f32 (precision-critical path)
            wm_sb = pers.tile([128, D], f32)
            nc.sync.dma_start(wm_sb[:], wm_d[:])
            wmt = pers.tile([128, D], f32)
            tpm = ps_z.tile([128, 128], f32, tag="z")
            nc.tensor.transpose(tpm[:], wm_sb[:], ident_f[:])
            nc.vector.tensor_copy(wmt[:], tpm[:])

            m_parts = pers.tile([128, NT], f32)
            hn_scr = dram.tile([128, N_SH], bf16)
            m_in = dram.tile([128], f32)
            m_out = dram.tile([128], f32, addr_space="Shared")

            # ---- pass 1 ----------------------------------------------------
            for i in range(NT):
                sl = slice(T * i, T * (i + 1))
                x_bf = io.tile([128, T], bf16)
                nc.gpsimd.dma_start(x_bf[:].rearrange("p (b d) -> p b d", b=NB),
                                    x_re[i])
                he_bf = io.tile([128, T], bf16)
                nc.gpsimd.dma_start(he_bf[:].rearrange("p (b d) -> p b d", b=NB),
                                    he_re[i])
                b_f = io.tile([K, T], f32)
                nc.sync.dma_start(b_f[:], bond_d[:, sl])
                b_bf = io.tile([K, T], bf16)
                nc.vector.tensor_copy(b_bf[:], b_f[:])

                # transpose 128x128 blocks via regular matmul (x_blk.T @ I):
                # keeps the PE HAM-warm and batches 4 blocks into one PSUM
                # bank so a single DVE copy drains it.
                xt_ps = ps_t.tile([128, T], f32, tag="tr")
                het_ps = ps_t.tile([128, T], f32, tag="tr")
                for b in range(NB):
                    bsl = slice(128 * b, 128 * (b + 1))
                    nc.tensor.matmul(xt_ps[:, bsl], x_bf[:, bsl], ident_bf[:],
                                     start=True, stop=True,
                                     skip_group_check=True)
                    nc.tensor.matmul(het_ps[:, bsl], he_bf[:, bsl], ident_bf[:],
                                     start=True, stop=True,
                                     skip_group_check=True)
                xt_bf = io.tile([128, T], bf16)
                nc.vector.tensor_copy(xt_bf[:], xt_ps[:])
                het_bf = io.tile([128, T], bf16)
                nc.vector.tensor_copy(het_bf[:], het_ps[:])

                z_ps = ps_z.tile([128, T], f32, tag="z")
                nc.tensor.matmul(z_ps[:], w1t[:, 0:128], xt_bf[:],
                                 start=True, stop=False)
                nc.tensor.matmul(z_ps[:], w1t[:, 128:256], het_bf[:],
                                 start=False, stop=True)

                wb_ps = ps_w.tile([128, T], f32, tag="wb")
                nc.tensor.matmul(wb_ps[:], ones32[:], b_bf[:],
                                 start=True, stop=True)

                hn_bf = io.tile([128, T], bf16)
                nc.scalar.activation(hn_bf[:], z_ps[:], AF.Relu, bias=b1_col[:])

                prod = io.tile([128, T], bf16)
                nc.vector.tensor_tensor(prod[:], hn_bf[:], wb_ps[:], ALU.mult)
                junk2 = io.tile([128, T], bf16)
                nc.scalar.activation(junk2[:], prod[:], AF.Copy,
                                     accum_out=m_parts[:, i:i + 1])

                nc.sync.dma_start(hn_scr[:, sl], hn_bf[:])

            # ---- m all-reduce + c ----------------------------------------
            import concourse.mybir as mybir_  # noqa
            m_col = pers.tile([128, 1], f32)
            nc.vector.reduce_sum(m_col[:], m_parts[:], axis=mybir.AxisListType.X)
            nc.sync.dma_start(m_in[:], m_col[:, 0])
            nc.gpsimd.collective_compute(
                "AllReduce", ALU.add,
                replica_groups=[list(range(CORES))],
                ins=[m_in[:].opt()], outs=[m_out[:].opt()])
            m_sb = pers.tile([128, 1], f32)
            nc.sync.dma_start(m_sb[:, 0], m_out[:])

            c_ps = ps_z.tile([128, 1], f32, tag="z")
            nc.tensor.matmul(c_ps[:], wmt[:], m_sb[:], start=True, stop=True)
            c_col = pers.tile([128, 1], f32)
            nc.vector.tensor_tensor(c_col[:], c_ps[:], b2_col[:], ALU.add)

            c_rps = ps_w.tile([1, 128], f32, tag="wb")
            nc.tensor.transpose(c_rps[:], c_col[:], ident_f[:])
            c_row = pers.tile([1, 128], f32)
            nc.vector.tensor_copy(c_row[:], c_rps[:])
            cb_ps = ps_z.tile([128, 128], f32, tag="z")
            nc.tensor.matmul(cb_ps[:], ones1[:], c_row[:], start=True, stop=True)
            c_bc = pers.tile([128, 128], f32)
            nc.vector.tensor_copy(c_bc[:], cb_ps[:])

            # ---- pass 2 ----------------------------------------------------
            c_rep = c_bc[:].rearrange("p (one d) -> p one d", one=1) \
                           .broadcast_to([128, NB, 128])
            for i in range(NT):
                sl = slice(T * i, T * (i + 1))
                hd_bf = io.tile([128, T], bf16)
                nc.sync.dma_start(hd_bf[:], hn_scr[:, sl])
                # transpose [d,t] -> [t,d] on the (otherwise idle) PE; the
                # DVE add drains PSUM and applies the broadcast message c.
                ht_ps = ps_t.tile([128, T], f32, tag="tr")
                for b in range(NB):
                    bsl = slice(128 * b, 128 * (b + 1))
                    nc.tensor.matmul(ht_ps[:, bsl], hd_bf[:, bsl], ident_bf[:],
                                     start=True, stop=True,
                                     skip_group_check=True)
                hf = io.tile([128, T], f32)
                nc.vector.tensor_tensor(
                    hf[:].rearrange("p (b d) -> p b d", b=NB),
                    ht_ps[:].rearrange("p (b d) -> p b d", b=NB),
                    c_rep, ALU.add)
                ho = io.tile([128, T], f32)
                nc.scalar.activation(ho[:], hf[:], AF.Relu)
                nc.scalar.dma_start(h_re[i],
                                    ho[:].rearrange("p (b d) -> p b d", b=NB))

    nc.compile()
    return nc


def _get_nc():
    if "nc" not in _cache:
        _cache["nc"] = _build()
    return _cache["nc"]


def _ensure_ntff_hook():
    """Register the axon NTFF profile hook if the image's antenv lacks it."""
    import types
    try:
        import antenv.axon_hooks  # noqa: F401
        return
    except ImportError:
        pass
    try:
        import antenv
        from trn_agent_boot.trn_boot import _ntff_profile_via_ctypes
        mod = types.ModuleType("antenv.axon_hooks")
        _h = {"hook": None}
        mod.set_axon_ntff_profile_hook = lambda h: _h.__setitem__("hook", h)
        mod.get_axon_ntff_profile_hook = lambda: _h["hook"]
        sys.modules["antenv.axon_hooks"] = mod
        antenv.axon_hooks = mod
        hook = _ntff_profile_via_ctypes("/opt/axon/libaxon_pjrt.so")
        if hook is not None:
            mod.set_axon_ntff_profile_hook(hook)
    except Exception:
        pass


def kernel(**inputs):
    global last_results
    from concourse.bass_utils import run_bass_kernel_spmd

    x = np.ascontiguousarray(np.asarray(inputs["x"], dtype=np.float32))
    he = np.ascontiguousarray(np.asarray(inputs["h_e"], dtype=np.float32))
    bond = np.ascontiguousarray(np.asarray(inputs["bond_n"], dtype=np.float32))
    wi = np.ascontiguousarray(np.asarray(inputs["W_i_w"], dtype=np.float32))
    bi = np.ascontiguousarray(np.asarray(inputs["W_i_b"], dtype=np.float32))
    wm = np.ascontiguousarray(np.asarray(inputs["W_m_w"], dtype=np.float32))
    bm = np.ascontiguousarray(np.asarray(inputs["W_m_b"], dtype=np.float32))

    n = x.shape[0]
    pad = N_PAD - n
    xp = np.concatenate([x, np.zeros((pad, D), np.float32)], 0)
    hep = np.concatenate([he, np.zeros((pad, D), np.float32)], 0)
    bondp = np.concatenate([bond, np.zeros((K, pad), np.float32)], 1)

    in_maps = []
    for c in range(CORES):
        sl = slice(c * N_SH, (c + 1) * N_SH)
        in_maps.append({
            "x": xp[sl],
            "h_e": hep[sl],
            "bond_n": np.ascontiguousarray(bondp[:, sl]),
            "W_i_w": wi, "W_i_b": bi, "W_m_w": wm, "W_m_b": bm,
        })

    nc = _get_nc()
    trace = os.environ.get("BASS_KERNEL_TRACE", "0") == "1"
    if trace:
        _ensure_ntff_hook()
    res = run_bass_kernel_spmd(nc, in_maps, core_ids=list(range(CORES)),
                               trace=trace)
    last_results = res
    out = np.concatenate([r["h"] for r in res.results], 0)[:n]
    return np.ascontiguousarray(out)



# revision 11
# speedup vs baseline: 1.3412x; 1.3412x over previous
"""DMPNNConv kernel for 8 Trainium2 NeuronCores.

  h_n = relu([x ; h_e] @ W_i_w.T + W_i_b)          [N, D]
  m   = einsum('kn,nd->d', bond_n, h_n)            [D]
  h   = relu(h_n + m @ W_m_w.T + W_m_b)            [N, D]

Sharding: N (edge dim) split 8 ways; weights replicated; single [D]
all-reduce of the message m between the two passes.

v2 design (vs the DRAM-scratch baseline):
  - host marshals x/h_e to feature-major bf16 [D, N_sh] and bond to
    bf16, so the device streams large contiguous DMA runs (4 KB/part)
    with no SWDGE cast and no PE transposes.
  - h_n stays RESIDENT in SBUF ([128, 63488] bf16 = 124 KB/partition),
    eliminating the 32 MB/core DRAM scratch round-trip.
  - pass 1 per 512-col tile: z = W1a.T@x + W1b.T@he (PSUM), ACT relu+
    bias -> resident hn, w broadcast = ones32.T@bond (PE), one DVE
    tensor_tensor_reduce for the m partial.
  - [D] message all-reduced between passes; c = W_m m + b2 on PE.
  - pass 2: relu(hn + c) per-partition bias, bf16 out, round-robined
    over ACT/DVE/GPSIMD; host upcasts/transposes the [D, N] output.
"""

import os
import sys

sys.path.insert(0, "/opt/trn_rl_repo")

import numpy as np

N, D, K = 500000, 128, 32
CORES = 8
T = 512                       # columns per compute tile (one PSUM bank)
NT = 124                      # tiles per core
N_SH = NT * T                 # 63488 padded tokens per core
N_PAD = CORES * N_SH          # 507904
CHUNK = 2048                  # columns per DMA chunk (4 KB runs)
NCH = N_SH // CHUNK           # 31
TPC = CHUNK // T              # tiles per chunk = 4

USE_ALLGATHER = os.environ.get("BASS_M_ALLGATHER", "1") == "1"

_cache = {}
last_results = None


def _build(nt=NT, chunk=CHUNK, use_allgather=None):
    import concourse.bass as bass  # noqa: F401
    import concourse.bacc as bacc
    import concourse.tile as tile
    import concourse.mybir as mybir

    n_sh = nt * T
    nch = n_sh // chunk
    tpc = chunk // T
    assert nch * chunk == n_sh
    if use_allgather is None:
        use_allgather = USE_ALLGATHER

    f32 = mybir.dt.float32
    bf16 = mybir.dt.bfloat16
    AF = mybir.ActivationFunctionType
    ALU = mybir.AluOpType

    nc = bacc.Bacc("TRN2", target_bir_lowering=False, debug=False,
                   num_devices=CORES)

    xT_d = nc.dram_tensor("xT", [D, n_sh], bf16, kind="ExternalInput").ap()
    heT_d = nc.dram_tensor("heT", [D, n_sh], bf16, kind="ExternalInput").ap()
    bond_d = nc.dram_tensor("bond", [K, n_sh], bf16, kind="ExternalInput").ap()
    w1t_d = nc.dram_tensor("w1t", [D, 2 * D], bf16, kind="ExternalInput").ap()
    b1_d = nc.dram_tensor("b1", [D], f32, kind="ExternalInput").ap()
    b2_d = nc.dram_tensor("b2", [D], f32, kind="ExternalInput").ap()
    wmT_d = nc.dram_tensor("wmT", [D, D], f32, kind="ExternalInput").ap()
    h_d = nc.dram_tensor("h", [D, n_sh], bf16, kind="ExternalOutput").ap()

    with tile.TileContext(nc) as tc:
        import contextlib
        ctx = contextlib.ExitStack()
        with ctx:
            pers = ctx.enter_context(tc.tile_pool(name="pers", bufs=1))
            iox = ctx.enter_context(tc.tile_pool(name="iox", bufs=3))
            ioh = ctx.enter_context(tc.tile_pool(name="ioh", bufs=3))
            iob = ctx.enter_context(tc.tile_pool(name="iob", bufs=3))
            ioo = ctx.enter_context(tc.tile_pool(name="ioo", bufs=4))
            ps_z = ctx.enter_context(tc.tile_pool(name="ps_z", bufs=2, space="PSUM"))
            ps_w = ctx.enter_context(tc.tile_pool(name="ps_w", bufs=2, space="PSUM"))
            ps_c = ctx.enter_context(tc.tile_pool(name="ps_c", bufs=1, space="PSUM"))
            dram = ctx.enter_context(tc.tile_pool(name="dram", bufs=1, space="DRAM"))

            # ---- one-time setup -------------------------------------------
            w1t = pers.tile([D, 2 * D], bf16)       # [j, d] halves: x | he
            nc.sync.dma_start(w1t[:], w1t_d[:])
            b1_col = pers.tile([D, 1], f32)
            nc.sync.dma_start(b1_col[:, 0], b1_d[:])
            b2_col = pers.tile([D, 1], f32)
            nc.sync.dma_start(b2_col[:, 0], b2_d[:])
            wmt = pers.tile([D, D], f32)            # [d', d] = W_m.T
            nc.sync.dma_start(wmt[:], wmT_d[:])
            ones32 = pers.tile([K, D], bf16)
            nc.gpsimd.memset(ones32[:], 1.0)

            hn_all = pers.tile([D, n_sh], bf16)     # resident h_n, 124 KB/part
            m_parts = pers.tile([D, nch], f32)
            junk = pers.tile([D, T], bf16)

            m_in = dram.tile([D], f32)
            if use_allgather:
                m_gath = dram.tile([CORES * D], f32, addr_space="Shared")
            else:
                m_out = dram.tile([D], f32, addr_space="Shared")

            # ---- pass 1 ----------------------------------------------------
            for ch in range(nch):
                csl = slice(chunk * ch, chunk * (ch + 1))
                xc = iox.tile([D, chunk], bf16)
                nc.sync.dma_start(xc[:], xT_d[:, csl])
                hc = ioh.tile([D, chunk], bf16)
                nc.scalar.dma_start(hc[:], heT_d[:, csl])
                bc = iob.tile([K, chunk], bf16)
                nc.sync.dma_start(bc[:], bond_d[:, csl])

                prod = ioo.tile([D, chunk], bf16, tag="prod")
                for t in range(tpc):
                    i = tpc * ch + t
                    tsl = slice(T * t, T * (t + 1))
                    gsl = slice(T * i, T * (i + 1))
                    z_ps = ps_z.tile([D, T], f32, tag="z")
                    nc.tensor.matmul(z_ps[:], w1t[:, 0:D], xc[:, tsl],
                                     start=True, stop=False)
                    nc.tensor.matmul(z_ps[:], w1t[:, D:2 * D], hc[:, tsl],
                                     start=False, stop=True)
                    wb_ps = ps_w.tile([D, T], f32, tag="wb")
                    nc.tensor.matmul(wb_ps[:], ones32[:], bc[:, tsl],
                                     start=True, stop=True)

                    nc.scalar.activation(hn_all[:, gsl], z_ps[:], AF.Relu,
                                         bias=b1_col[:])
                    nc.vector.tensor_tensor(prod[:, tsl], hn_all[:, gsl],
                                            wb_ps[:], ALU.mult)
                nc.vector.tensor_reduce(
                    m_parts[:, ch:ch + 1], prod[:],
                    mybir.AxisListType.X, ALU.add)

            # ---- m all-reduce + c ----------------------------------------
            m_col = pers.tile([D, 1], f32)
            nc.vector.reduce_sum(m_col[:], m_parts[:], axis=mybir.AxisListType.X)
            nc.sync.dma_start(m_in[:], m_col[:, 0])
            m_sb = pers.tile([D, 1], f32)
            if use_allgather:
                nc.gpsimd.collective_compute(
                    "AllGather", ALU.bypass,
                    replica_groups=[list(range(CORES))],
                    ins=[m_in[:].opt()], outs=[m_gath[:].opt()])
                m_g = pers.tile([D, CORES], f32)
                nc.sync.dma_start(
                    m_g[:], m_gath[:].rearrange("(r p) -> p r", p=D))
                nc.vector.reduce_sum(m_sb[:], m_g[:], axis=mybir.AxisListType.X)
            else:
                nc.gpsimd.collective_compute(
                    "AllReduce", ALU.add,
                    replica_groups=[list(range(CORES))],
                    ins=[m_in[:].opt()], outs=[m_out[:].opt()])
                nc.sync.dma_start(m_sb[:, 0], m_out[:])

            c_ps = ps_c.tile([D, 1], f32, tag="c")
            nc.tensor.matmul(c_ps[:], wmt[:], m_sb[:], start=True, stop=True)
            c_col = pers.tile([D, 1], f32)
            nc.vector.tensor_tensor(c_col[:], c_ps[:], b2_col[:], ALU.add)

            # ---- pass 2: h = relu(hn + c), bf16 out -----------------------
            for ch in range(nch):
                csl = slice(chunk * ch, chunk * (ch + 1))
                ob = ioo.tile([D, chunk], bf16)
                r = ch % 4
                if os.environ.get("BASS_PASS2_ACT", "0") == "1":
                    r = 0
                if r == 0:
                    nc.scalar.activation(ob[:], hn_all[:, csl], AF.Relu,
                                         bias=c_col[:])
                elif r == 2:
                    nc.gpsimd.tensor_scalar(ob[:], hn_all[:, csl],
                                            c_col[:], 0.0,
                                            ALU.add, ALU.max)
                else:
                    nc.vector.tensor_scalar(ob[:], hn_all[:, csl],
                                            c_col[:], 0.0,
                                            ALU.add, ALU.max)
                if ch % 2 == 0:
                    nc.sync.dma_start(h_d[:, csl], ob[:])
                else:
                    nc.scalar.dma_start(h_d[:, csl], ob[:])

    nc.compile()
    return nc


def _get_nc():
    if "nc" not in _cache:
        _cache["nc"] = _build()
    return _cache["nc"]


def _ensure_ntff_hook():
    """Register the axon NTFF profile hook if the image's antenv lacks it."""
    import types
    try:
        import antenv.axon_hooks  # noqa: F401
        return
    except ImportError:
        pass
    try:
        import antenv
        from trn_agent_boot.trn_boot import _ntff_profile_via_ctypes
        mod = types.ModuleType("antenv.axon_hooks")
        _h = {"hook": None}
        mod.set_axon_ntff_profile_hook = lambda h: _h.__setitem__("hook", h)
        mod.get_axon_ntff_profile_hook = lambda: _h["hook"]
        sys.modules["antenv.axon_hooks"] = mod
        antenv.axon_hooks = mod
        hook = _ntff_profile_via_ctypes("/opt/axon/libaxon_pjrt.so")
        if hook is not None:
            mod.set_axon_ntff_profile_hook(hook)
    except Exception:
        pass


def kernel(**inputs):
    global last_results
    import ml_dtypes
    from concourse.bass_utils import run_bass_kernel_spmd

    bf = np.dtype(ml_dtypes.bfloat16)

    x = np.asarray(inputs["x"], dtype=np.float32)
    he = np.asarray(inputs["h_e"], dtype=np.float32)
    bond = np.asarray(inputs["bond_n"], dtype=np.float32)
    wi = np.asarray(inputs["W_i_w"], dtype=np.float32)
    bi = np.ascontiguousarray(np.asarray(inputs["W_i_b"], dtype=np.float32))
    wm = np.asarray(inputs["W_m_w"], dtype=np.float32)
    bm = np.ascontiguousarray(np.asarray(inputs["W_m_b"], dtype=np.float32))

    n = x.shape[0]
    # feature-major weight marshalling (lhsT layouts)
    w1t = np.empty((D, 2 * D), bf)
    w1t[:, 0:D] = wi[:, 0:D].T.astype(bf)
    w1t[:, D:2 * D] = wi[:, D:2 * D].T.astype(bf)
    wmT = np.ascontiguousarray(wm.T)

    xT, heT = x.T, he.T        # views
    in_maps = []
    for c in range(CORES):
        lo = c * N_SH
        hi = min(n, lo + N_SH)
        v = max(0, hi - lo)
        xc = np.zeros((D, N_SH), bf)
        hc = np.zeros((D, N_SH), bf)
        bc = np.zeros((K, N_SH), bf)
        if v > 0:
            xc[:, :v] = xT[:, lo:hi]
            hc[:, :v] = heT[:, lo:hi]
            bc[:, :v] = bond[:, lo:hi]
        in_maps.append({
            "xT": xc, "heT": hc, "bond": bc,
            "w1t": w1t, "b1": bi, "b2": bm, "wmT": wmT,
        })

    nc = _get_nc()
    trace = os.environ.get("BASS_KERNEL_TRACE", "0") == "1"
    if trace:
        _ensure_ntff_hook()
    res = run_bass_kernel_spmd(nc, in_maps, core_ids=list(range(CORES)),
                               trace=trace)
    last_results = res
    hT = np.concatenate([r["h"] for r in res.results], axis=1)[:, :n]
    return hT.T.astype(np.float32, order="C")


# revision 12
# speedup vs baseline: 1.9949x; 1.4874x over previous
"""DMPNNConv kernel for 8 Trainium2 NeuronCores.

  h_n = relu([x ; h_e] @ W_i_w.T + W_i_b)          [N, D]
  m   = einsum('kn,nd->d', bond_n, h_n)            [D]
  h   = relu(h_n + m @ W_m_w.T + W_m_b)            [N, D]

Sharding: N (edge dim) split 8 ways; weights replicated; single [D]
all-gather (+local sum) of the message m between the two passes.

Design (vs the DRAM-scratch baseline):
  - host marshals x/h_e to feature-major fp8e4 (w1 pre-scaled x16,
    undone by the ACT scale) and bond to fp8, so the device streams
    large contiguous DMA runs with no SWDGE cast and no PE transposes.
  - z = W1.T @ [x;he] in ONE DoubleRow matmul (256-contract fp8).
  - h_n stays RESIDENT in SBUF ([128, 63488] bf16 = 124 KB/partition),
    eliminating the 32 MB/core DRAM scratch round-trip.
  - per 512-col tile: ACT relu+bias+scale -> resident hn; w broadcast
    = ones32.T@bond (PE); DVE prod = hn*wb; one batched DVE reduce per
    2048-col chunk accumulates the m partials.
  - [D] message all-gathered between passes; c = W_m m + b2 on PE.
  - pass 2: h = relu(hn + c) alternating ACT (per-partition bias) and
    DVE (stride-0 broadcast add + relu), bf16 out on two DMA queues;
    host upcasts/transposes the [D, N] output.
"""

import os
import sys

sys.path.insert(0, "/opt/trn_rl_repo")

import numpy as np

N, D, K = 500000, 128, 32
CORES = 8
T = 512                       # columns per compute tile (one PSUM bank)
NT = 124                      # tiles per core
N_SH = NT * T                 # 63488 padded tokens per core
N_PAD = CORES * N_SH          # 507904
CHUNK = 2048                  # columns per DMA chunk
NCH = N_SH // CHUNK           # 31
TPC = CHUNK // T              # tiles per chunk = 4
WSCALE = 16.0                 # fp8 weight pre-scale (undone in ACT)

USE_FP8 = os.environ.get("BASS_FP8", "1") == "1"

_cache = {}
last_results = None


def _build(nt=NT, chunk=CHUNK, fp8=None):
    import concourse.bass as bass  # noqa: F401
    import concourse.bacc as bacc
    import concourse.tile as tile
    import concourse.mybir as mybir

    n_sh = nt * T
    nch = n_sh // chunk
    tpc = chunk // T
    assert nch * chunk == n_sh
    if fp8 is None:
        fp8 = USE_FP8

    f32 = mybir.dt.float32
    bf16 = mybir.dt.bfloat16
    f8 = mybir.dt.float8e4
    in_dt = f8 if fp8 else bf16
    AF = mybir.ActivationFunctionType
    ALU = mybir.AluOpType
    red_mode = os.environ.get("BASS_RED", "reduce")

    nc = bacc.Bacc("TRN2", target_bir_lowering=False, debug=False,
                   num_devices=CORES)

    xT_d = nc.dram_tensor("xT", [D, n_sh], in_dt, kind="ExternalInput").ap()
    heT_d = nc.dram_tensor("heT", [D, n_sh], in_dt, kind="ExternalInput").ap()
    bond_d = nc.dram_tensor("bond", [K, n_sh], in_dt, kind="ExternalInput").ap()
    w1t_d = nc.dram_tensor("w1t", [D, 2 * D], in_dt, kind="ExternalInput").ap()
    b1_d = nc.dram_tensor("b1", [D], f32, kind="ExternalInput").ap()
    b2_d = nc.dram_tensor("b2", [D], f32, kind="ExternalInput").ap()
    wmT_d = nc.dram_tensor("wmT", [D, D], f32, kind="ExternalInput").ap()
    h_d = nc.dram_tensor("h", [D, n_sh], bf16, kind="ExternalOutput").ap()

    with tile.TileContext(nc) as tc:
        import contextlib
        ctx = contextlib.ExitStack()
        with ctx:
            pers = ctx.enter_context(tc.tile_pool(name="pers", bufs=1))
            iox = ctx.enter_context(tc.tile_pool(name="iox", bufs=3))
            iob = ctx.enter_context(tc.tile_pool(name="iob", bufs=3))
            ioo = ctx.enter_context(tc.tile_pool(name="ioo", bufs=4))
            ps_z = ctx.enter_context(tc.tile_pool(name="ps_z", bufs=2, space="PSUM"))
            ps_w = ctx.enter_context(tc.tile_pool(name="ps_w", bufs=2, space="PSUM"))
            ps_c = ctx.enter_context(tc.tile_pool(name="ps_c", bufs=1, space="PSUM"))
            dram = ctx.enter_context(tc.tile_pool(name="dram", bufs=1, space="DRAM"))

            # ---- one-time setup -------------------------------------------
            # stationary weights: fp8 DoubleRow layout [j, 2, d] (x|he), or
            # bf16 [j, d] halves side by side.
            w1t = pers.tile([D, 2 * D], in_dt)
            nc.sync.dma_start(w1t[:], w1t_d[:])
            b1_col = pers.tile([D, 1], f32)
            nc.sync.dma_start(b1_col[:, 0], b1_d[:])
            b2_col = pers.tile([D, 1], f32)
            nc.sync.dma_start(b2_col[:, 0], b2_d[:])
            wmt = pers.tile([D, D], f32)            # [d', d] = W_m.T
            nc.sync.dma_start(wmt[:], wmT_d[:])
            ones32 = pers.tile([K, D], in_dt)
            nc.gpsimd.memset(ones32[:], 1.0)

            hn_all = pers.tile([D, n_sh], bf16)     # resident h_n
            m_parts = pers.tile([D, nch], f32)
            junk = pers.tile([D, chunk], bf16)

            m_in = dram.tile([D], f32)
            m_gath = dram.tile([CORES * D], f32, addr_space="Shared")

            if fp8:
                w1dr = w1t[:].rearrange("j (two d) -> j two d", two=2)
                dr_mode = mybir.MatmulPerfMode.DoubleRow
            zscale = 1.0 / WSCALE if fp8 else 1.0

            # ---- pass 1 ----------------------------------------------------
            for ch in range(nch):
                csl = slice(chunk * ch, chunk * (ch + 1))
                xc = iox.tile([D, 2, chunk], in_dt, tag="xhe")
                nc.sync.dma_start(xc[:, 0], xT_d[:, csl])
                nc.scalar.dma_start(xc[:, 1], heT_d[:, csl])
                bc = iob.tile([K, chunk], in_dt)
                nc.sync.dma_start(bc[:], bond_d[:, csl])

                prod = ioo.tile([D, chunk], bf16, tag="prod")
                for t in range(tpc):
                    tsl = slice(T * t, T * (t + 1))
                    gsl = slice(T * (tpc * ch + t), T * (tpc * ch + t + 1))
                    z_ps = ps_z.tile([D, T], f32, tag="z")
                    if fp8:
                        nc.tensor.matmul(z_ps[:], w1dr, xc[:, :, tsl],
                                         start=True, stop=True,
                                         perf_mode=dr_mode)
                    else:
                        nc.tensor.matmul(z_ps[:], w1t[:, 0:D], xc[:, 0, tsl],
                                         start=True, stop=False)
                        nc.tensor.matmul(z_ps[:], w1t[:, D:2 * D],
                                         xc[:, 1, tsl],
                                         start=False, stop=True)
                    wb_ps = ps_w.tile([D, T], f32, tag="wb")
                    nc.tensor.matmul(wb_ps[:], ones32[:], bc[:, tsl],
                                     start=True, stop=True)

                    nc.scalar.activation(hn_all[:, gsl], z_ps[:], AF.Relu,
                                         bias=b1_col[:], scale=zscale)
                    nc.vector.tensor_tensor(prod[:, tsl], hn_all[:, gsl],
                                            wb_ps[:], ALU.mult)
                if red_mode == "ts":
                    nc.vector.tensor_scalar(
                        junk[:], prod[:], 1.0, 0.0, ALU.mult, ALU.add,
                        accum_out=m_parts[:, ch:ch + 1])
                else:
                    nc.vector.tensor_reduce(
                        m_parts[:, ch:ch + 1], prod[:],
                        mybir.AxisListType.X, ALU.add)

            # ---- m all-gather + local sum + c -----------------------------
            m_col = pers.tile([D, 1], f32)
            nc.vector.reduce_sum(m_col[:], m_parts[:], axis=mybir.AxisListType.X)
            nc.sync.dma_start(m_in[:], m_col[:, 0])
            nc.gpsimd.collective_compute(
                "AllGather", ALU.bypass,
                replica_groups=[list(range(CORES))],
                ins=[m_in[:].opt()], outs=[m_gath[:].opt()])
            m_g = pers.tile([D, CORES], f32)
            nc.sync.dma_start(m_g[:], m_gath[:].rearrange("(r p) -> p r", p=D))
            m_sb = pers.tile([D, 1], f32)
            nc.vector.reduce_sum(m_sb[:], m_g[:], axis=mybir.AxisListType.X)

            c_ps = ps_c.tile([D, 1], f32, tag="c")
            nc.tensor.matmul(c_ps[:], wmt[:], m_sb[:], start=True, stop=True)
            c_col = pers.tile([D, 1], f32)
            nc.vector.tensor_tensor(c_col[:], c_ps[:], b2_col[:], ALU.add)
            c_bf = pers.tile([D, 1], bf16)
            nc.vector.tensor_copy(c_bf[:], c_col[:])

            # ---- pass 2: h = relu(hn + c), bf16 out -----------------------
            c_bc = c_bf[:].broadcast_to([D, chunk])
            for ch in range(nch):
                csl = slice(chunk * ch, chunk * (ch + 1))
                ob = ioo.tile([D, chunk], bf16, tag="ob")
                if ch % 2 == 0:
                    nc.scalar.activation(ob[:], hn_all[:, csl], AF.Relu,
                                         bias=c_col[:])
                else:
                    nc.vector.tensor_tensor(ob[:], hn_all[:, csl], c_bc,
                                            ALU.add)
                    nc.vector.tensor_relu(ob[:], ob[:])
                if ch % 2 == 0:
                    nc.sync.dma_start(h_d[:, csl], ob[:])
                else:
                    nc.scalar.dma_start(h_d[:, csl], ob[:])

    nc.compile()
    return nc


def _get_nc():
    if "nc" not in _cache:
        _cache["nc"] = _build()
    return _cache["nc"]


def _ensure_ntff_hook():
    """Register the axon NTFF profile hook if the image's antenv lacks it."""
    import types
    try:
        import antenv.axon_hooks  # noqa: F401
        return
    except ImportError:
        pass
    try:
        import antenv
        from trn_agent_boot.trn_boot import _ntff_profile_via_ctypes
        mod = types.ModuleType("antenv.axon_hooks")
        _h = {"hook": None}
        mod.set_axon_ntff_profile_hook = lambda h: _h.__setitem__("hook", h)
        mod.get_axon_ntff_profile_hook = lambda: _h["hook"]
        sys.modules["antenv.axon_hooks"] = mod
        antenv.axon_hooks = mod
        hook = _ntff_profile_via_ctypes("/opt/axon/libaxon_pjrt.so")
        if hook is not None:
            mod.set_axon_ntff_profile_hook(hook)
    except Exception:
        pass


def marshal_inputs(inputs, n_sh=N_SH, fp8=None):
    """Host-side marshalling: shard + feature-major layout + dtype cast."""
    import ml_dtypes
    if fp8 is None:
        fp8 = USE_FP8
    in_np = np.dtype(ml_dtypes.float8_e4m3) if fp8 else \
        np.dtype(ml_dtypes.bfloat16)
    ws = WSCALE if fp8 else 1.0

    x = np.asarray(inputs["x"], dtype=np.float32)
    he = np.asarray(inputs["h_e"], dtype=np.float32)
    bond = np.asarray(inputs["bond_n"], dtype=np.float32)
    wi = np.asarray(inputs["W_i_w"], dtype=np.float32)
    bi = np.ascontiguousarray(np.asarray(inputs["W_i_b"], dtype=np.float32))
    wm = np.asarray(inputs["W_m_w"], dtype=np.float32)
    bm = np.ascontiguousarray(np.asarray(inputs["W_m_b"], dtype=np.float32))

    n = x.shape[0]
    w1t = np.empty((D, 2 * D), in_np)
    if fp8:
        # DoubleRow layout [j, 2, d]: slot 0 = x half, slot 1 = h_e half
        w1t3 = w1t.reshape(D, 2, D)
        w1t3[:, 0, :] = (wi[:, 0:D].T * ws).astype(in_np)
        w1t3[:, 1, :] = (wi[:, D:2 * D].T * ws).astype(in_np)
    else:
        w1t[:, 0:D] = wi[:, 0:D].T.astype(in_np)
        w1t[:, D:2 * D] = wi[:, D:2 * D].T.astype(in_np)
    wmT = np.ascontiguousarray(wm.T)

    xT, heT = x.T, he.T
    in_maps = []
    for c in range(CORES):
        lo = c * n_sh
        hi = min(n, lo + n_sh)
        v = max(0, hi - lo)
        xc = np.zeros((D, n_sh), in_np)
        hc = np.zeros((D, n_sh), in_np)
        bc = np.zeros((K, n_sh), in_np)
        if v > 0:
            xc[:, :v] = xT[:, lo:hi]
            hc[:, :v] = heT[:, lo:hi]
            bc[:, :v] = bond[:, lo:hi]
        in_maps.append({
            "xT": xc, "heT": hc, "bond": bc,
            "w1t": w1t, "b1": bi, "b2": bm, "wmT": wmT,
        })
    return in_maps, n


def kernel(**inputs):
    global last_results
    from concourse.bass_utils import run_bass_kernel_spmd

    in_maps, n = marshal_inputs(inputs)
    nc = _get_nc()
    trace = os.environ.get("BASS_KERNEL_TRACE", "0") == "1"
    if trace:
        _ensure_ntff_hook()
    res = run_bass_kernel_spmd(nc, in_maps, core_ids=list(range(CORES)),
                               trace=trace)
    last_results = res
    hT = np.concatenate([r["h"] for r in res.results], axis=1)[:, :n]
    return hT.T.astype(np.float32, order="C")
